# revision 19
# baseline (speedup 1.0000x reference)
"""Grouped GEMM (MoE routing) Trainium2 kernel.

Strategy: tensor-parallel shard of the output N dim across 8 NeuronCores.
Every core sees all T=8192 tokens and a 512-wide slice of every expert's
weights, so per-core work is identical regardless of segment sizes and a
single SPMD program (with the segment boundaries baked in as compile-time
constants) runs on all 8 cores.

Per core:  out_t[n, t] = sum_k w_t[e(t), k, n] * a_t[k, t]

Inputs and outputs are cast to bf16 on the host (rel err ~3e-3, far
under the 2e-2 gate), cutting HBM traffic to ~109MB/core, below the
~280GB/s effective per-core DMA roofline for the 437us compute span.
All DMAs are laid out so each SBUF partition line is one contiguous
HBM run.  a-block loads ride the sync HWDGE queue; weight loads +
output stores ride the scalar HWDGE queue.  Experts are processed in
descending segment-length order and the next expert's weight chunks
are paced across the current run's blocks, so prefetches never burst
against the a-stream at the shared HBM port.  A short burst of dummy
matmuls on scratch SBUF warms the PE clock-gate (HAM 1.2->2.4GHz)
during the initial loads.

Matmul mapping: stationary lhsT = w tile [k=128, n=128], moving rhs =
a tile [k=128, tok<=512] in bf16, PSUM out [n=128, tok<=512] fp32,
accumulated over the 32 k-chunks.  Compute floor/core = T*K*NS/(128*128)
cycles @2.4GHz = 437us; measured ~465us (framework preamble + teardown
account for most of the difference).
"""

import numpy as np
import ml_dtypes

import concourse.bacc as bacc
import concourse.bass as bass
import concourse.mybir as mybir
import concourse.tile as tile
from concourse.bass_utils import run_bass_kernel_spmd

NC = 8          # NeuronCores
P = 128         # partitions
TB = 512        # max token block (PSUM bank = 512 fp32)

BF16 = ml_dtypes.bfloat16

LAST_RESULT = {}


def _token_blocks(seg_starts, seg_ends):
    """Split each segment into even pieces of <=512 tokens."""
    blocks = []  # (tstart, tlen, active_expert_idx)
    for widx, (s, t) in enumerate(zip(seg_starts, seg_ends)):
        ln = t - s
        npieces = max(1, -(-ln // TB))
        base, rem = divmod(ln, npieces)
        p = s
        for i in range(npieces):
            L = base + (1 if i < rem else 0)
            if L > 0:
                blocks.append((p, L, widx))
                p += L
    return blocks


WCH = 8         # ko per weight/a DMA chunk (4 chunks of 8KB+/partition)


def _build_program(T, K, NS, EA, blocks):
    f32 = mybir.dt.float32
    bf16 = mybir.dt.bfloat16
    KO = K // P
    NB = NS // P
    NCH = KO // WCH

    CTA = sum(KO * L for (_, L, _) in blocks)
    CTO = sum(NB * L for (_, L, _) in blocks)

    # group consecutive same-expert blocks into runs
    runs = []
    for blk in blocks:
        if runs and runs[-1][0] == blk[2]:
            runs[-1][1].append(blk)
        else:
            runs.append((blk[2], [blk]))

    nc = bacc.Bacc(None, target_bir_lowering=False)
    ab = nc.declare_dram_parameter("ab", [P, CTA], bf16, isOutput=False)
    wb = nc.declare_dram_parameter("wb", [EA, P, KO, NS], bf16, isOutput=False)
    ot = nc.declare_dram_parameter("ot", [P, CTO], bf16, isOutput=True)

    with tile.TileContext(nc) as tc:
        with (
            tc.tile_pool(name="wpool", bufs=3) as wpool,
            tc.tile_pool(name="apool", bufs=3) as apool,
            tc.tile_pool(name="opool", bufs=2) as opool,
            tc.tile_pool(name="psum", bufs=2, space=bass.MemorySpace.PSUM) as psum_pool,
        ):
            def load_w_range(wt, widx, s, e):
                nc.scalar.dma_start(
                    out=wt[:, s:e, :],
                    in_=wb[widx, :, s:e, :])

            # ko ranges for the leading loads: very fine at first so the
            # first matmul can start ~1us after the queues open
            FINE = [(0, 2), (2, 4), (4, 8), (8, 16), (16, 24), (24, 32)]
            COARSE = [(0, 16), (16, 32)]

            # PE pre-warm: dummy matmuls on scratch SBUF with no DMA deps
            # run during the initial load, so the HAM clock-gate opens
            # (1.2->2.4GHz takes ~3.4us of sustained PE work) before the
            # first real matmul issues.
            warm_w = wpool.tile([P, P], bf16, tag="warm", name="warm_w")
            warm_a = apool.tile([P, 192], bf16, tag="warm", name="warm_a")
            nc.vector.memset(warm_w[:, :], 0)
            nc.vector.memset(warm_a[:, :], 0)
            warm_ps = psum_pool.tile([P, 1, 192], f32, tag="ps", name="warm_ps",
                                     padded_shape=[P, NB, TB])
            for _ in range(24):
                nc.tensor.matmul(warm_ps[:, 0, :], warm_w[:, :], warm_a[:, :],
                                 start=True, stop=True)

            off_a = 0
            off_o = 0
            w_next = wpool.tile([P, KO, NS], bf16, tag="w", name="w_tile")
            for (s, e) in FINE:
                load_w_range(w_next, runs[0][0], s, e)
            for ri, (widx, rblocks) in enumerate(runs):
                w_tile = w_next
                nbk = len(rblocks)
                if ri + 1 < len(runs):
                    w_next = wpool.tile([P, KO, NS], bf16, tag="w", name="w_tile")
                for bi, (ts, L, _) in enumerate(rblocks):
                    a_tile = apool.tile([P, KO * L], bf16, tag="a", name="a_tile",
                                        padded_shape=[P, KO * TB])
                    for (s, e) in (FINE if (ri == 0 and bi == 0) else COARSE):
                        nc.sync.dma_start(
                            out=a_tile[:, s * L:e * L],
                            in_=ab[:, off_a + s * L:off_a + e * L])
                    ptile = psum_pool.tile([P, NB, L], f32, tag="ps", name="ps",
                                           padded_shape=[P, NB, TB])
                    for ko in range(KO):
                        for nb in range(NB):
                            nc.tensor.matmul(
                                ptile[:, nb, :],
                                w_tile[:, ko, nb * P:(nb + 1) * P],
                                a_tile[:, ko * L:(ko + 1) * L],
                                start=(ko == 0),
                                stop=(ko == KO - 1),
                            )
                    o_tile = opool.tile([P, NB * L], bf16, tag="o", name="o_tile",
                                        padded_shape=[P, NB * TB])
                    for nb in range(NB):
                        nc.vector.tensor_copy(o_tile[:, nb * L:(nb + 1) * L],
                                              ptile[:, nb, :])
                    nc.scalar.dma_start(out=ot[:, off_o:off_o + NB * L],
                                        in_=o_tile[:, :])
                    off_a += KO * L
                    off_o += NB * L
                    # pace the next expert's weight chunks across this run's
                    # blocks so the prefetch never bursts against the a-stream
                    if ri + 1 < len(runs):
                        c0 = bi * NCH // nbk
                        c1 = (bi + 1) * NCH // nbk
                        for c in range(c0, c1):
                            load_w_range(w_next, runs[ri + 1][0],
                                         c * WCH, (c + 1) * WCH)
    nc.compile()
    return nc


def kernel(a, b, c, seg_indptr, weight_indices, batch_size, **_):
    T, K = a.shape
    E, N, K2 = b.shape
    assert K == K2
    NS = N // NC
    KO = K // P
    NB = NS // P

    seg = np.asarray(seg_indptr).astype(np.int64)
    widx_arr = np.asarray(weight_indices).astype(np.int64)
    segs = [(int(seg[e]), int(seg[e + 1]), int(widx_arr[e]))
            for e in range(int(batch_size)) if seg[e + 1] > seg[e]]
    # process longest segments first: every expert switch is then covered by
    # a long compute run, hiding the next weight load entirely
    segs.sort(key=lambda s: s[0] - s[1])
    seg_starts = [s for s, _, _ in segs]
    seg_ends = [t for _, t, _ in segs]
    experts = [w for _, _, w in segs]
    EA = len(segs)
    blocks = _token_blocks(seg_starts, seg_ends)

    # a -> [P, KO, T] bf16 (partition-major k layout), then pack blocks so
    # each block is a [P, KO*L] slab with 32KB-contiguous partition lines.
    a = np.ascontiguousarray(a, dtype=np.float32)
    at_full = a.T.reshape(KO, P, T).transpose(1, 0, 2).astype(BF16)  # [P,KO,T]
    CTA = sum(KO * L for (_, L, _) in blocks)
    ab_np = np.empty((P, CTA), dtype=BF16)
    off = 0
    for (ts, L, _) in blocks:
        ab_np[:, off:off + KO * L] = at_full[:, :, ts:ts + L].reshape(P, KO * L)
        off += KO * L

    # weights: full [E_active, P, KO, N] bf16 once, slice per core.
    wt_full = np.empty((EA, P, KO, N), dtype=BF16)
    for ei, e in enumerate(experts):
        wt_full[ei] = b[e].T.reshape(KO, P, N).transpose(1, 0, 2)

    in_maps = []
    for j in range(NC):
        w = np.ascontiguousarray(wt_full[:, :, :, j * NS:(j + 1) * NS])
        in_maps.append({"ab": ab_np, "wb": w})

    nc = _build_program(T, K, NS, EA, blocks)

    import os
    trace = bool(int(os.environ.get("BASS_KERNEL_TRACE", "0")))
    res = run_bass_kernel_spmd(nc, in_maps, list(range(NC)), trace=trace)
    LAST_RESULT["exec_time_ns"] = res.exec_time_ns
    LAST_RESULT["results"] = res

    out_t = np.empty((N, T), dtype=np.float32)
    for j in range(NC):
        otj = np.asarray(res.results[j]["ot"]).astype(np.float32)  # [P, CTO]
        off = 0
        for (ts, L, _) in blocks:
            blk = otj[:, off:off + NB * L].reshape(P, NB, L)
            out_t[j * NS:(j + 1) * NS, ts:ts + L] = (
                blk.transpose(1, 0, 2).reshape(NS, L))
            off += NB * L
    return np.ascontiguousarray(out_t.T)
